# revision 22
# baseline (speedup 1.0000x reference)
"""Trainium2 Bass kernel for CurlVectorField.

curl(psi) where psi = W3 tanh(W2 tanh(W1 x + b1) + b2) + b3, x in R^3,
N = 524288 points. Data-parallel over 8 NeuronCores.

Math (per point, h1 = tanh(W1x+b1), s = h1^2, h2 = tanh(W2h1+b2),
D2 = 1-h2^2):
  curl_c = sum_h D2[h]*(cst_c[h] - (B_c @ s)[h])
  B_c[h,k] = W3[c2,h]W2[h,k]W1[k,c1] - W3[c1,h]W2[h,k]W1[k,c2]
  cst_c[h] = sum_k B_c[h,k]

Layout: 2 points per column, H=64 -> 128 partitions. Per tile (512 cols
= 1024 points), with engines balanced and emission software-pipelined
(tile t emits Y-matmuls for t-1 and reduction matmuls for t-2 so no
engine waits on same-tile dependencies):

  PE  : p1(t) z1-matmul | Y(t-1) 3x psumY | p2(t) z2-matmul |
        red(t-2) 3x gsel + 1x gcst           (~1.92us)
  Act : tanh1(t) -> fp16, tanh2(t) -> fp16, yq copy per 3-tile group
  Pool: S2 = S*S (stt), Q = T2*T2 (stt)      (SBUF only - no PSUM!)
  DVE : V(t-1) = psumY3 * D2 (one fused 3-bank op), D2(t) = 1-Q (4x fp16)
  SP  : batched input DMA (per 8 tiles), output DMAs per group

PSUM banks: p12 shared 1 + psumY3 (3-bank tile) x2 bufs + psum5q 1 = 8.
psum5q packs 3 tiles' 6-row outputs in one bank via reverse strips
(rows 64:70, 32:38, 0:6) with zero-padded stationaries.
Host packs x as (6, NSH/2) [even xyz; odd xyz] and unpacks yt6 (6, NSH/2).
"""

import os
import sys
from contextlib import ExitStack

import numpy as np

sys.path.insert(0, "/opt/trn_rl_repo")

import concourse.bass as bass
import concourse.bacc as bacc
import concourse.tile as tile
from concourse import mybir
from concourse.bass import broadcast_tensor_aps
from concourse.bass_utils import run_bass_kernel_spmd

N_CORES = 8
NPTS = 524288
NSH = NPTS // N_CORES          # 65536 points per core
NSH2 = NSH // 2                # 32768 columns per core
H = 64
TILE_N = 512
NT = NSH2 // TILE_N            # 64 iterations per core
XCHUNK = 16                    # input tiles per DMA batch
QA = 192                       # Q-square columns done on Act (rest on Pool)
GROUP = 6                      # tiles per psum5q strip group
GRB = tuple(16 * (GROUP - 1 - r) for r in range(GROUP))   # rowbase per strip
GM = tuple(rb + 6 for rb in GRB)                          # matmul M per strip
GOFF = tuple(int(np.cumsum((0,) + GM)[r]) for r in range(GROUP))
GW = sum(GM)                                              # packed gsel width
GSLC = tuple(slice(GOFF[r], GOFF[r] + GM[r]) for r in range(GROUP))

F32 = mybir.dt.float32
F32R = mybir.dt.float32r
F16 = mybir.dt.float16


def _build_program():
    nc = bacc.Bacc(
        "TRN2",
        target_bir_lowering=False,
        debug=False,
        num_devices=N_CORES,
    )

    xt6 = nc.dram_tensor("xt6", [6, NSH2], F32R, kind="ExternalInput").ap()
    w1bd = nc.dram_tensor("w1bd", [6, 128], F32R, kind="ExternalInput").ap()
    b1d = nc.dram_tensor("b1d", [128, 1], F32, kind="ExternalInput").ap()
    w2bd = nc.dram_tensor("w2bd", [128, 128], F16, kind="ExternalInput").ap()
    b2d = nc.dram_tensor("b2d", [128, 1], F32, kind="ExternalInput").ap()
    bB = nc.dram_tensor("bB", [128, 3, 128], F16, kind="ExternalInput").ap()
    gsel = nc.dram_tensor("gsel", [128, 3, GW], F32R, kind="ExternalInput").ap()
    gcst = nc.dram_tensor("gcst", [128, GW], F16, kind="ExternalInput").ap()
    yt6 = nc.dram_tensor("yt6", [6, NSH2], F32, kind="ExternalOutput").ap()

    with tile.TileContext(nc) as tc, ExitStack() as ctx:
        consts = ctx.enter_context(tc.tile_pool(name="consts", bufs=1))
        xin = ctx.enter_context(tc.tile_pool(name="xin", bufs=2))
        sb = ctx.enter_context(tc.tile_pool(name="sb", bufs=3))
        vb = ctx.enter_context(tc.tile_pool(name="vb", bufs=3))
        pp12 = ctx.enter_context(tc.tile_pool(name="pp12", bufs=2, space="PSUM"))
        ppy = ctx.enter_context(tc.tile_pool(name="ppy", bufs=2, space="PSUM"))
        ppc = ctx.enter_context(tc.tile_pool(name="ppc", bufs=1, space="PSUM"))
        pp5 = ctx.enter_context(tc.tile_pool(name="pp5", bufs=1, space="PSUM"))

        w1bd_s = consts.tile([6, 128], F32R)
        b1d_s = consts.tile([128, 1], F32)
        w2bd_s = consts.tile([128, 128], F16)
        b2d_s = consts.tile([128, 1], F32)
        bB_s = consts.tile([128, 3, 128], F16)
        gsel_s = consts.tile([128, 3, GW], F32R)
        gcst_s = consts.tile([128, GW], F16)
        for dst, src in (
            (w1bd_s, w1bd), (b1d_s, b1d), (w2bd_s, w2bd), (b2d_s, b2d),
            (bB_s, bB), (gsel_s, gsel), (gcst_s, gcst),
        ):
            nc.sync.dma_start(out=dst, in_=src)

        # per-logical-tile live tensors, indexed mod small history
        hist = {}
        psum5q = None
        pend = []
        xt_t = None

        def sl_of(t):
            return slice(t * TILE_N, (t + 1) * TILE_N)

        xt_next = None
        for t in range(NT + 2):
            if t < NT:
                # prefetch: chunk 0 at t=0, chunk k+1 mid-way through chunk k
                if t == 0:
                    xt_t = xin.tile([6, XCHUNK * TILE_N], F32R)
                    nc.sync.dma_start(out=xt_t,
                                      in_=xt6[:, 0:XCHUNK * TILE_N])
                elif t % XCHUNK == XCHUNK // 2 and t + XCHUNK // 2 < NT:
                    base = (t // XCHUNK + 1) * XCHUNK * TILE_N
                    xt_next = xin.tile([6, XCHUNK * TILE_N], F32R)
                    nc.sync.dma_start(
                        out=xt_next,
                        in_=xt6[:, base:base + XCHUNK * TILE_N])
                elif t % XCHUNK == 0 and t > 0:
                    xt_t = xt_next
                xsl = slice((t % XCHUNK) * TILE_N, (t % XCHUNK + 1) * TILE_N)

                # PE: z1
                psum1 = pp12.tile([128, TILE_N], F32, tag="p12")
                nc.tensor.matmul(psum1, w1bd_s[:, :], xt_t[:, xsl],
                                 start=True, stop=True)
                # Act: tanh1 -> fp16
                ST = sb.tile([128, 2, TILE_N], F16, tag="ST")
                nc.scalar.activation(ST[:, 0, :], psum1[:, :],
                                     mybir.ActivationFunctionType.Tanh,
                                     bias=b1d_s[:, :])
                hist[t] = {"ST": ST}

            # PE: psumY c=0,1 for t-1 (needs S2(t-1), ready since last iter)
            if 0 <= t - 1 < NT:
                h1 = hist[t - 1]
                psumYab = ppy.tile([128, 2, TILE_N], F32, tag="psumYab")
                for c in range(2):
                    nc.tensor.matmul(psumYab[:, c, :], bB_s[:, c, :],
                                     h1["SQ"][:, 0, :], start=True, stop=True)
                h1["psumYab"] = psumYab

            if t < NT:
                h = hist[t]
                ST = h["ST"]
                # Pool: S2 = S*S
                SQ = sb.tile([128, 2, TILE_N], F16, tag="SQ")
                nc.gpsimd.tensor_mul(SQ[:, 0, :], ST[:, 0, :], ST[:, 0, :])
                h["SQ"] = SQ

                # PE: z2
                psum2 = pp12.tile([128, TILE_N], F32, tag="p12")
                nc.tensor.matmul(psum2, w2bd_s[:, :], ST[:, 0, :],
                                 start=True, stop=True)
                # Act: tanh2 -> fp16; Q = T2*T2 split Act/Pool to balance
                nc.scalar.activation(ST[:, 1, :], psum2[:, :],
                                     mybir.ActivationFunctionType.Tanh,
                                     bias=b2d_s[:, :])
                nc.scalar.activation(SQ[:, 1, 0:QA], ST[:, 1, 0:QA],
                                     mybir.ActivationFunctionType.Square)
                nc.gpsimd.tensor_mul(SQ[:, 1, QA:TILE_N],
                                     ST[:, 1, QA:TILE_N],
                                     ST[:, 1, QA:TILE_N])

            # PE: reduction for t-2; reverse-strip packing shares one PSUM
            # bank across 3 tiles (group iter r -> rows 64:70 / 32:38 / 0:6)
            if 0 <= t - 2 < NT:
                tau = t - 2
                h2 = hist[tau]
                if not pend:
                    psum5q = pp5.tile([128, TILE_N], F32, tag="psum5q")
                r = len(pend)
                gslc = GSLC[r]
                m = GM[r]
                rowbase = GRB[r]
                last = (r == GROUP - 1) or (tau == NT - 1)
                for c in range(3):
                    nc.tensor.matmul(psum5q[0:m, :], gsel_s[:, c, gslc],
                                     h2["V"][:, c, :],
                                     start=(r == 0 and c == 0), stop=False,
                                     skip_group_check=True)
                nc.tensor.matmul(psum5q[0:m, :], gcst_s[:, gslc],
                                 h2["D2"][:, 0, :],
                                 start=False, stop=last,
                                 skip_group_check=True)
                pend.append((rowbase, sl_of(tau)))
                del hist[tau]

                if last:
                    yq = vb.tile([128, TILE_N], F32, tag="yq")
                    nc.scalar.copy(yq[:, :], psum5q[:, :])
                    for (rb, ssl) in pend:
                        nc.sync.dma_start(out=yt6[:, ssl],
                                          in_=yq[rb:rb + 6, :])
                    pend = []

            # PE: psumY c=2 for t-1, LAST in PE order so the single ppc
            # bank has a full period of slack before V2 reads it
            if 0 <= t - 1 < NT:
                h1 = hist[t - 1]
                psumYc = ppc.tile([128, TILE_N], F32, tag="psumYc")
                nc.tensor.matmul(psumYc[:, :], bB_s[:, 2, :],
                                 h1["SQ"][:, 0, :], start=True, stop=True)
                h1["psumYc"] = psumYc

                # DVE: V01(t-1) fused over the 2-bank pair, then V2(t-1)
                V = vb.tile([128, 3, TILE_N], F32R, tag="V")
                in0, in1 = broadcast_tensor_aps(h1["psumYab"][:, :, :],
                                                h1["D2"][:, :, :])
                nc.vector.tensor_mul(V[:, 0:2, :], in0, in1)
                nc.vector.tensor_mul(V[:, 2, :], h1["psumYc"][:, :],
                                     h1["D2"][:, 0, :])
                h1["V"] = V

            if t < NT:
                h = hist[t]
                # DVE: D2 = 1 - Q (4x fp16 mode), after the V ops in DVE order
                D2 = sb.tile([128, 1, TILE_N], F16, tag="D2")
                nc.vector.tensor_scalar(D2[:, 0, :], h["SQ"][:, 1, :],
                                        -1.0, 1.0,
                                        mybir.AluOpType.mult,
                                        mybir.AluOpType.add)
                h["D2"] = D2

    nc.compile()
    return nc


_NC_CACHE = None


def _get_program():
    global _NC_CACHE
    if _NC_CACHE is None:
        _NC_CACHE = _build_program()
    return _NC_CACHE


def _host_weights(W1, b1, W2, b2, W3):
    W1 = np.asarray(W1, np.float32)
    W2 = np.asarray(W2, np.float32)
    W3 = np.asarray(W3, np.float32)
    b1 = np.asarray(b1, np.float32)
    b2 = np.asarray(b2, np.float32)
    M = np.einsum("hk,kj->jhk", W2, W1)          # M_j = W2 * W1[:,j]
    B = np.stack([
        W3[2][:, None] * M[1] - W3[1][:, None] * M[2],
        W3[0][:, None] * M[2] - W3[2][:, None] * M[0],
        W3[1][:, None] * M[0] - W3[0][:, None] * M[1],
    ]).astype(np.float32)                         # (3, H, H)
    cst = B.sum(axis=2)                           # (3, H)

    Z = np.zeros((64, 64), np.float32)
    bd = lambda A: np.block([[A, Z], [Z, A]]).astype(np.float32)

    w1bd = np.zeros((6, 128), np.float32)
    w1bd[0:3, 0:64] = W1.T
    w1bd[3:6, 64:128] = W1.T

    gsel6 = np.zeros((3, 128, 6), np.float32)
    for c in range(3):
        gsel6[c, 0:64, c] = 1.0
        gsel6[c, 64:128, 3 + c] = 1.0
    gcst6 = np.zeros((128, 6), np.float32)
    for c in range(3):
        gcst6[0:64, c] = cst[c]
        gcst6[64:128, 3 + c] = cst[c]
    # packed reverse-strip variants, GROUP strips of 6 rows each
    gsel = np.zeros((3, 128, GW), np.float32)
    gcst = np.zeros((128, GW), np.float32)
    for r in range(GROUP):
        off = GOFF[r]
        rb = GRB[r]
        gsel[:, :, off + rb:off + rb + 6] = gsel6
        gcst[:, off + rb:off + rb + 6] = gcst6

    c_ = np.ascontiguousarray
    return {
        "w1bd": c_(w1bd),
        "b1d": c_(np.concatenate([b1, b1])[:, None]),
        "w2bd": bd(W2.T).astype(np.float16),
        "b2d": c_(np.concatenate([b2, b2])[:, None]),
        "bB": c_(np.stack([bd(-B[c].T) for c in range(3)], axis=1)
                 ).astype(np.float16),
        "gsel": c_(gsel.transpose(1, 0, 2)),
        "gcst": gcst.astype(np.float16),
    }


def kernel(x, W1, b1, W2, b2, W3, b3, _want_trace=False):
    x = np.asarray(x, np.float32)
    wts = _host_weights(W1, b1, W2, b2, W3)

    in_maps = []
    for ci in range(N_CORES):
        xs = x[ci * NSH:(ci + 1) * NSH]                       # (NSH, 3)
        xt6 = np.ascontiguousarray(
            xs.reshape(NSH2, 2, 3).transpose(1, 2, 0).reshape(6, NSH2))
        m = {"xt6": xt6}
        m.update(wts)
        in_maps.append(m)

    nc = _get_program()
    res = None
    for attempt in range(3):
        try:
            res = run_bass_kernel_spmd(nc, in_maps, list(range(N_CORES)),
                                       trace=_want_trace)
            break
        except Exception as e:
            # Axon-tunneled NeuronCores occasionally report a transient
            # NRT_EXEC_UNIT_UNRECOVERABLE; a retry on the same devices
            # consistently succeeds.
            if attempt == 2 or "UNRECOVERABLE" not in str(e).upper():
                raise
            import time
            time.sleep(10)
    outs = []
    for ci in range(N_CORES):
        yt6 = res.results[ci]["yt6"]                          # (6, NSH2)
        y = yt6.reshape(2, 3, NSH2).transpose(2, 0, 1).reshape(NSH, 3)
        outs.append(y)
    out = np.ascontiguousarray(np.concatenate(outs, axis=0)).astype(np.float32)
    if _want_trace:
        return out, res
    return out


# revision 23
# speedup vs baseline: 1.0030x; 1.0030x over previous
"""Trainium2 Bass kernel for CurlVectorField.

curl(psi) where psi = W3 tanh(W2 tanh(W1 x + b1) + b2) + b3, x in R^3,
N = 524288 points. Data-parallel over 8 NeuronCores.

Math (per point, h1 = tanh(W1x+b1), s = h1^2, h2 = tanh(W2h1+b2),
D2 = 1-h2^2):
  curl_c = sum_h D2[h]*(cst_c[h] - (B_c @ s)[h])
  B_c[h,k] = W3[c2,h]W2[h,k]W1[k,c1] - W3[c1,h]W2[h,k]W1[k,c2]
  cst_c[h] = sum_k B_c[h,k]

Layout: 2 points per column, H=64 -> 128 partitions. Per tile (512 cols
= 1024 points), with engines balanced and emission software-pipelined
(tile t emits Y-matmuls for t-1 and reduction matmuls for t-2 so no
engine waits on same-tile dependencies):

  PE  : p1(t) z1-matmul | Y(t-1) 3x psumY | p2(t) z2-matmul |
        red(t-2) 3x gsel + 1x gcst           (~1.92us)
  Act : tanh1(t) -> fp16, tanh2(t) -> fp16, yq copy per 3-tile group
  Pool: S2 = S*S (stt), Q = T2*T2 (stt)      (SBUF only - no PSUM!)
  DVE : V(t-1) = psumY3 * D2 (one fused 3-bank op), D2(t) = 1-Q (4x fp16)
  SP  : batched input DMA (per 8 tiles), output DMAs per group

PSUM banks: p12 shared 1 + psumY3 (3-bank tile) x2 bufs + psum5q 1 = 8.
psum5q packs 3 tiles' 6-row outputs in one bank via reverse strips
(rows 64:70, 32:38, 0:6) with zero-padded stationaries.
Host packs x as (6, NSH/2) [even xyz; odd xyz] and unpacks yt6 (6, NSH/2).
"""

import os
import sys
from contextlib import ExitStack

import numpy as np

sys.path.insert(0, "/opt/trn_rl_repo")

import concourse.bass as bass
import concourse.bacc as bacc
import concourse.tile as tile
from concourse import mybir
from concourse.bass import broadcast_tensor_aps
from concourse.bass_utils import run_bass_kernel_spmd

N_CORES = 8
NPTS = 524288
NSH = NPTS // N_CORES          # 65536 points per core
NSH2 = NSH // 2                # 32768 columns per core
H = 64
TILE_N = 512
NT = NSH2 // TILE_N            # 64 iterations per core
XCHUNK = 8                     # input tiles per DMA batch
QA = 192                       # Q-square columns done on Act (rest on Pool)
GROUP = 6                      # tiles per psum5q strip group
GRB = tuple(16 * (GROUP - 1 - r) for r in range(GROUP))   # rowbase per strip
GM = tuple(rb + 6 for rb in GRB)                          # matmul M per strip
GOFF = tuple(int(np.cumsum((0,) + GM)[r]) for r in range(GROUP))
GW = sum(GM)                                              # packed gsel width
GSLC = tuple(slice(GOFF[r], GOFF[r] + GM[r]) for r in range(GROUP))

F32 = mybir.dt.float32
F32R = mybir.dt.float32r
F16 = mybir.dt.float16


def _build_program():
    nc = bacc.Bacc(
        "TRN2",
        target_bir_lowering=False,
        debug=False,
        num_devices=N_CORES,
    )

    xt6 = nc.dram_tensor("xt6", [6, NSH2], F32R, kind="ExternalInput").ap()
    w1bd = nc.dram_tensor("w1bd", [6, 128], F32R, kind="ExternalInput").ap()
    b1d = nc.dram_tensor("b1d", [128, 1], F32, kind="ExternalInput").ap()
    w2bd = nc.dram_tensor("w2bd", [128, 128], F16, kind="ExternalInput").ap()
    b2d = nc.dram_tensor("b2d", [128, 1], F32, kind="ExternalInput").ap()
    bB = nc.dram_tensor("bB", [128, 3, 128], F16, kind="ExternalInput").ap()
    gsel = nc.dram_tensor("gsel", [128, 3, GW], F32R, kind="ExternalInput").ap()
    gcst = nc.dram_tensor("gcst", [128, GW], F16, kind="ExternalInput").ap()
    yt6 = nc.dram_tensor("yt6", [6, NSH2], F32, kind="ExternalOutput").ap()

    with tile.TileContext(nc) as tc, ExitStack() as ctx:
        consts = ctx.enter_context(tc.tile_pool(name="consts", bufs=1))
        xin = ctx.enter_context(tc.tile_pool(name="xin", bufs=2))
        sb = ctx.enter_context(tc.tile_pool(name="sb", bufs=3))
        vb = ctx.enter_context(tc.tile_pool(name="vb", bufs=3))
        pp12 = ctx.enter_context(tc.tile_pool(name="pp12", bufs=2, space="PSUM"))
        ppy = ctx.enter_context(tc.tile_pool(name="ppy", bufs=2, space="PSUM"))
        ppc = ctx.enter_context(tc.tile_pool(name="ppc", bufs=1, space="PSUM"))
        pp5 = ctx.enter_context(tc.tile_pool(name="pp5", bufs=1, space="PSUM"))

        w1bd_s = consts.tile([6, 128], F32R)
        b1d_s = consts.tile([128, 1], F32)
        w2bd_s = consts.tile([128, 128], F16)
        b2d_s = consts.tile([128, 1], F32)
        bB_s = consts.tile([128, 3, 128], F16)
        gsel_s = consts.tile([128, 3, GW], F32R)
        gcst_s = consts.tile([128, GW], F16)
        for dst, src in (
            (w1bd_s, w1bd), (b1d_s, b1d), (w2bd_s, w2bd), (b2d_s, b2d),
            (bB_s, bB), (gsel_s, gsel), (gcst_s, gcst),
        ):
            nc.sync.dma_start(out=dst, in_=src)

        # per-logical-tile live tensors, indexed mod small history
        hist = {}
        psum5q = None
        pend = []
        xt_t = None

        def sl_of(t):
            return slice(t * TILE_N, (t + 1) * TILE_N)

        xt_next = None
        for t in range(NT + 2):
            if t < NT:
                # prefetch: chunk 0 at t=0, chunk k+1 mid-way through chunk k
                if t == 0:
                    xt_t = xin.tile([6, XCHUNK * TILE_N], F32R)
                    nc.sync.dma_start(out=xt_t,
                                      in_=xt6[:, 0:XCHUNK * TILE_N])
                elif t % XCHUNK == XCHUNK // 2 and t + XCHUNK // 2 < NT:
                    base = (t // XCHUNK + 1) * XCHUNK * TILE_N
                    xt_next = xin.tile([6, XCHUNK * TILE_N], F32R)
                    nc.sync.dma_start(
                        out=xt_next,
                        in_=xt6[:, base:base + XCHUNK * TILE_N])
                elif t % XCHUNK == 0 and t > 0:
                    xt_t = xt_next
                xsl = slice((t % XCHUNK) * TILE_N, (t % XCHUNK + 1) * TILE_N)

                # PE: z1
                psum1 = pp12.tile([128, TILE_N], F32, tag="p12")
                nc.tensor.matmul(psum1, w1bd_s[:, :], xt_t[:, xsl],
                                 start=True, stop=True)
                # Act: tanh1 -> fp16
                ST = sb.tile([128, 2, TILE_N], F16, tag="ST")
                nc.scalar.activation(ST[:, 0, :], psum1[:, :],
                                     mybir.ActivationFunctionType.Tanh,
                                     bias=b1d_s[:, :])
                hist[t] = {"ST": ST}

            # PE: psumY c=0,1 for t-1 (needs S2(t-1), ready since last iter)
            if 0 <= t - 1 < NT:
                h1 = hist[t - 1]
                psumYab = ppy.tile([128, 2, TILE_N], F32, tag="psumYab")
                for c in range(2):
                    nc.tensor.matmul(psumYab[:, c, :], bB_s[:, c, :],
                                     h1["SQ"][:, 0, :], start=True, stop=True)
                h1["psumYab"] = psumYab

            if t < NT:
                h = hist[t]
                ST = h["ST"]
                # Pool: S2 = S*S
                SQ = sb.tile([128, 2, TILE_N], F16, tag="SQ")
                nc.gpsimd.tensor_mul(SQ[:, 0, :], ST[:, 0, :], ST[:, 0, :])
                h["SQ"] = SQ

                # PE: z2
                psum2 = pp12.tile([128, TILE_N], F32, tag="p12")
                nc.tensor.matmul(psum2, w2bd_s[:, :], ST[:, 0, :],
                                 start=True, stop=True)
                # Act: tanh2 -> fp16; Q = T2*T2 split Act/Pool to balance
                nc.scalar.activation(ST[:, 1, :], psum2[:, :],
                                     mybir.ActivationFunctionType.Tanh,
                                     bias=b2d_s[:, :])
                nc.scalar.activation(SQ[:, 1, 0:QA], ST[:, 1, 0:QA],
                                     mybir.ActivationFunctionType.Square)
                nc.gpsimd.tensor_mul(SQ[:, 1, QA:TILE_N],
                                     ST[:, 1, QA:TILE_N],
                                     ST[:, 1, QA:TILE_N])

            # PE: reduction for t-2; reverse-strip packing shares one PSUM
            # bank across 3 tiles (group iter r -> rows 64:70 / 32:38 / 0:6)
            if 0 <= t - 2 < NT:
                tau = t - 2
                h2 = hist[tau]
                if not pend:
                    psum5q = pp5.tile([128, TILE_N], F32, tag="psum5q")
                r = len(pend)
                gslc = GSLC[r]
                m = GM[r]
                rowbase = GRB[r]
                last = (r == GROUP - 1) or (tau == NT - 1)
                for c in range(3):
                    nc.tensor.matmul(psum5q[0:m, :], gsel_s[:, c, gslc],
                                     h2["V"][:, c, :],
                                     start=(r == 0 and c == 0), stop=False,
                                     skip_group_check=True)
                nc.tensor.matmul(psum5q[0:m, :], gcst_s[:, gslc],
                                 h2["D2"][:, 0, :],
                                 start=False, stop=last,
                                 skip_group_check=True)
                pend.append((rowbase, sl_of(tau)))
                del hist[tau]

                if last:
                    yq = vb.tile([128, TILE_N], F32, tag="yq")
                    nc.scalar.copy(yq[:, :], psum5q[:, :])
                    for (rb, ssl) in pend:
                        nc.sync.dma_start(out=yt6[:, ssl],
                                          in_=yq[rb:rb + 6, :])
                    pend = []

            # PE: psumY c=2 for t-1, LAST in PE order so the single ppc
            # bank has a full period of slack before V2 reads it
            if 0 <= t - 1 < NT:
                h1 = hist[t - 1]
                psumYc = ppc.tile([128, TILE_N], F32, tag="psumYc")
                nc.tensor.matmul(psumYc[:, :], bB_s[:, 2, :],
                                 h1["SQ"][:, 0, :], start=True, stop=True)
                h1["psumYc"] = psumYc

                # DVE: V01(t-1) fused over the 2-bank pair, then V2(t-1)
                V = vb.tile([128, 3, TILE_N], F32R, tag="V")
                in0, in1 = broadcast_tensor_aps(h1["psumYab"][:, :, :],
                                                h1["D2"][:, :, :])
                nc.vector.tensor_mul(V[:, 0:2, :], in0, in1)
                nc.vector.tensor_mul(V[:, 2, :], h1["psumYc"][:, :],
                                     h1["D2"][:, 0, :])
                h1["V"] = V

            if t < NT:
                h = hist[t]
                # DVE: D2 = 1 - Q (4x fp16 mode), after the V ops in DVE order
                D2 = sb.tile([128, 1, TILE_N], F16, tag="D2")
                nc.vector.tensor_scalar(D2[:, 0, :], h["SQ"][:, 1, :],
                                        -1.0, 1.0,
                                        mybir.AluOpType.mult,
                                        mybir.AluOpType.add)
                h["D2"] = D2

    nc.compile()
    return nc


_NC_CACHE = None


def _get_program():
    global _NC_CACHE
    if _NC_CACHE is None:
        _NC_CACHE = _build_program()
    return _NC_CACHE


def _host_weights(W1, b1, W2, b2, W3):
    W1 = np.asarray(W1, np.float32)
    W2 = np.asarray(W2, np.float32)
    W3 = np.asarray(W3, np.float32)
    b1 = np.asarray(b1, np.float32)
    b2 = np.asarray(b2, np.float32)
    M = np.einsum("hk,kj->jhk", W2, W1)          # M_j = W2 * W1[:,j]
    B = np.stack([
        W3[2][:, None] * M[1] - W3[1][:, None] * M[2],
        W3[0][:, None] * M[2] - W3[2][:, None] * M[0],
        W3[1][:, None] * M[0] - W3[0][:, None] * M[1],
    ]).astype(np.float32)                         # (3, H, H)
    cst = B.sum(axis=2)                           # (3, H)

    Z = np.zeros((64, 64), np.float32)
    bd = lambda A: np.block([[A, Z], [Z, A]]).astype(np.float32)

    w1bd = np.zeros((6, 128), np.float32)
    w1bd[0:3, 0:64] = W1.T
    w1bd[3:6, 64:128] = W1.T

    gsel6 = np.zeros((3, 128, 6), np.float32)
    for c in range(3):
        gsel6[c, 0:64, c] = 1.0
        gsel6[c, 64:128, 3 + c] = 1.0
    gcst6 = np.zeros((128, 6), np.float32)
    for c in range(3):
        gcst6[0:64, c] = cst[c]
        gcst6[64:128, 3 + c] = cst[c]
    # packed reverse-strip variants, GROUP strips of 6 rows each
    gsel = np.zeros((3, 128, GW), np.float32)
    gcst = np.zeros((128, GW), np.float32)
    for r in range(GROUP):
        off = GOFF[r]
        rb = GRB[r]
        gsel[:, :, off + rb:off + rb + 6] = gsel6
        gcst[:, off + rb:off + rb + 6] = gcst6

    c_ = np.ascontiguousarray
    return {
        "w1bd": c_(w1bd),
        "b1d": c_(np.concatenate([b1, b1])[:, None]),
        "w2bd": bd(W2.T).astype(np.float16),
        "b2d": c_(np.concatenate([b2, b2])[:, None]),
        "bB": c_(np.stack([bd(-B[c].T) for c in range(3)], axis=1)
                 ).astype(np.float16),
        "gsel": c_(gsel.transpose(1, 0, 2)),
        "gcst": gcst.astype(np.float16),
    }


def kernel(x, W1, b1, W2, b2, W3, b3, _want_trace=False):
    x = np.asarray(x, np.float32)
    wts = _host_weights(W1, b1, W2, b2, W3)

    in_maps = []
    for ci in range(N_CORES):
        xs = x[ci * NSH:(ci + 1) * NSH]                       # (NSH, 3)
        xt6 = np.ascontiguousarray(
            xs.reshape(NSH2, 2, 3).transpose(1, 2, 0).reshape(6, NSH2))
        m = {"xt6": xt6}
        m.update(wts)
        in_maps.append(m)

    nc = _get_program()
    res = None
    for attempt in range(3):
        try:
            res = run_bass_kernel_spmd(nc, in_maps, list(range(N_CORES)),
                                       trace=_want_trace)
            break
        except Exception as e:
            # Axon-tunneled NeuronCores occasionally report a transient
            # NRT_EXEC_UNIT_UNRECOVERABLE; a retry on the same devices
            # consistently succeeds.
            if attempt == 2 or "UNRECOVERABLE" not in str(e).upper():
                raise
            import time
            time.sleep(10)
    outs = []
    for ci in range(N_CORES):
        yt6 = res.results[ci]["yt6"]                          # (6, NSH2)
        y = yt6.reshape(2, 3, NSH2).transpose(2, 0, 1).reshape(NSH, 3)
        outs.append(y)
    out = np.ascontiguousarray(np.concatenate(outs, axis=0)).astype(np.float32)
    if _want_trace:
        return out, res
    return out
